# revision 30
# baseline (speedup 1.0000x reference)
"""Attention-Jacobian kernel on 8 TRN2 NeuronCores (batch-sharded SPMD).

Full problem: query (16,256,64), keys (16,2048,64), values (16,2048,64)
-> out (16,256,64,64), out[b,q,i,j] = d attn_out[b,q,i] / d query[b,q,j]:
   scale * (sum_s a[q,s] v[s,i] k[s,j] - wv[q,i] wk[q,j])

Sharding: batch dim 16 -> 8 cores x 2 batches, pure data parallel.

Per-core algorithm (s-major, all heavy matmuls bf16 at N=512):
  - K^T/Q^T via DMA-xbar transposes of bf16 [K|K]/[Q|Q] dup tiles; scores
    run full-128 contraction (2x score absorbed into the exp scale)
  - exp on ACT -> ET bf16 (unnormalized; randn keeps scores ~N(0,1))
  - Z rides as a ones-column in the [V|K|1] rhs of the wv/wk accumulation;
    normalization folds into the PSUM->SBUF out-copy (ACT scale=SCALE/Z)
  - M[s, i*64+j] = V[s,i]*K[s,j] on DVE (pair-dup 2x_1P mode)
  - term1: PE c-major accumulation, lhsT = ET chunks, rhs = M chunks
  - term2: first two groups (PSUM bank first-use) via identity matmul of
    T2 = (-wvE/Z) x wkE; all later groups get T2 DVE-preloaded into PSUM
    and accumulate onto it with start=False
  - emission order prefix(b0) -> term1(b0,hq0) -> prefix(b1) -> rest, so
    the PE never queues exp-b1-gated work ahead of ready term1-b0 work
"""
import math
import numpy as np
import concourse.bass as bass
import concourse.tile as tile
from concourse import mybir
from concourse.masks import make_identity

FP32 = mybir.dt.float32
BF16 = mybir.dt.bfloat16
AF = mybir.ActivationFunctionType
ALU = mybir.AluOpType

NCORES = 8
B, Q, S, D = 16, 256, 2048, 64
BB = B // NCORES
SCALE = 1.0 / math.sqrt(D)

C = S // 128          # s-chunks (16)
T = Q // 128          # q-tiles (2)
NQ = 4                # i-quarters
IQ = D // NQ          # i per quarter (16)


def build(nc):
    from contextlib import ExitStack

    q_ext = nc.declare_dram_parameter("query", [BB, Q, D], FP32, isOutput=False)
    k_ext = nc.declare_dram_parameter("keys", [BB, S, D], FP32, isOutput=False)
    v_ext = nc.declare_dram_parameter("values", [BB, S, D], FP32, isOutput=False)
    out_ext = nc.declare_dram_parameter("out", [BB, Q, D * D], FP32, isOutput=True)

    with tile.TileContext(nc) as tc, ExitStack() as stack:
        ep = lambda name, bufs, **kw: stack.enter_context(
            tc.tile_pool(name=name, bufs=bufs, **kw))
        constp = ep("const", 1)
        kv32p = ep("kv32", 4)
        q32p = ep("q32", 2)
        vk1p = ep("vk1", 2)
        vk1wp = ep("vk1w", 2)
        vktp = ep("vkt", 2)
        qbpp = ep("qbp", 2)
        qtp = ep("qt", 2)
        etp = ep("et", 2)
        vdupp = ep("vdup", 2)
        wvkp = ep("wvk", 2)
        smallp = ep("small", 4)
        t2p = ep("t2", 2)
        mp = ep("m", 12)
        outsp = ep("outs", 6)

        ident32 = constp.tile([128, 128], FP32, tag="id32")
        make_identity(nc, ident32[:])
        ident16 = constp.tile([128, 128], BF16, tag="id16")
        nc.vector.tensor_copy(ident16[:], ident32[:])

        VK1, VK1W, VKT, QT, ET, VD = {}, {}, {}, {}, {}, {}
        WVP, WKP, RQ1 = {}, {}, {}

        # PSUM: scps(3 banks) + wvkps(part bank) + t1pspA(4 banks) coexist;
        # after pfx closes, t1pspB takes the freed banks.
        pfx = ExitStack()
        wmpsp = pfx.enter_context(tc.tile_pool(name="wmps", bufs=1, space="PSUM"))
        scpsp = pfx.enter_context(tc.tile_pool(name="scps", bufs=3, space="PSUM"))
        wvkpsp = pfx.enter_context(tc.tile_pool(name="wvkps", bufs=2, space="PSUM"))
        tpsp = pfx.enter_context(tc.tile_pool(name="tps", bufs=2, space="PSUM"))

        # ---------------- loads + casts + transposes ----------------
        # sync hwdge: q0, q1, k0a, k0b, qt0T, vkt0aT, vkt0bT, then staged
        # k1a, k1b, qt1T, vkt1T, then out DMAs. gpsimd swdge: v0, v1 only.
        CH = C // 2
        k32, v32, qq32, vk1s = {}, {}, {}, {}
        for b in range(BB):
            qq32[b] = q32p.tile([128, T * 64], FP32, tag="q32", name=f"qq{b}")
            k32[b] = kv32p.tile([128, C * 64], FP32, tag="k32", name=f"k32_{b}")
            v32[b] = kv32p.tile([128, C * 64], FP32, tag="v32", name=f"v32_{b}")
            kb = vk1p.tile([128, C * 128], BF16, tag="kb", name=f"kb{b}")
            vk1s[b] = kb
            VK1[b] = kb
            VKT[b] = vktp.tile([128, C * 128], BF16, tag="ktb", name=f"ktb{b}")
            QT[b] = qtp.tile([128, T * 128], BF16, tag="qt", name=f"qt{b}")
        qbps = {}
        with tc.high_priority():
            # HAM warmup on a memset tile: no DVE dependency; the warmup
            # PSUM tile shares the scps ring (same shape as pssc)
            z16 = constp.tile([128, 128], BF16, tag="z16")
            nc.gpsimd.memset(z16[:], 0.0)
            wm = wmpsp.tile([128, 128], FP32, tag="wm")
            for r in range(36):
                nc.tensor.matmul(wm[:], z16[:], z16[:], start=True, stop=True)
            for b in range(BB):
                nc.sync.dma_start(
                    qq32[b][:].rearrange("p (t d) -> p t d", t=T),
                    q_ext[b].rearrange("(t p) d -> p t d", p=128))
            for half in range(2):
                nc.sync.dma_start(
                    k32[0][:, half * CH * 64:(half + 1) * CH * 64]
                        .rearrange("p (c d) -> p c d", c=CH),
                    k_ext[0][half * CH * 128:(half + 1) * CH * 128]
                        .rearrange("(c p) d -> p c d", p=128))
            nc.gpsimd.dma_start(
                v32[0][:].rearrange("p (c d) -> p c d", c=C),
                v_ext[0].rearrange("(c p) d -> p c d", p=128))
            qbp = qbpp.tile([128, T * 128], BF16, tag="qbp", name="qbp0")
            for t in range(T):
                for hh in range(2):
                    nc.vector.tensor_copy(
                        qbp[:, t * 128 + hh * 64:t * 128 + (hh + 1) * 64],
                        qq32[0][:, t * 64:(t + 1) * 64])
            qbps[0] = qbp
            kbv0 = vk1s[0][:].rearrange("p (c w) -> p c w", c=C)
            for half in range(2):
                for hh in range(2):
                    nc.vector.tensor_copy(
                        kbv0[:, half * CH:(half + 1) * CH,
                             hh * 64:(hh + 1) * 64],
                        k32[0][:, half * CH * 64:(half + 1) * CH * 64]
                            .rearrange("p (c d) -> p c d", c=CH))
        nc.gpsimd.dma_start(
            v32[1][:].rearrange("p (c d) -> p c d", c=C),
            v_ext[1].rearrange("(c p) d -> p c d", p=128))
        # stage ~9us: qbp1 + qt1T
        with tc.tile_wait_until(0.009):
            qbp = qbpp.tile([128, T * 128], BF16, tag="qbp", name="qbp1")
            for t in range(T):
                for hh in range(2):
                    nc.vector.tensor_copy(
                        qbp[:, t * 128 + hh * 64:t * 128 + (hh + 1) * 64],
                        qq32[1][:, t * 64:(t + 1) * 64])
            qbps[1] = qbp
        # stage ~12us: k1 on sync (queues behind the b0 transposes), vk1w b0
        with tc.tile_wait_until(0.012):
            for half in range(2):
                nc.sync.dma_start(
                    k32[1][:, half * CH * 64:(half + 1) * CH * 64]
                        .rearrange("p (c d) -> p c d", c=CH),
                    k_ext[1][half * CH * 128:(half + 1) * CH * 128]
                        .rearrange("(c p) d -> p c d", p=128))
            vk1w = vk1wp.tile([128, C * 132], BF16, tag="vk1w", name="vk1w0")
            vk1wv = vk1w[:].rearrange("p (c w) -> p c w", c=C)
            nc.vector.tensor_copy(
                vk1wv[:, :, 0:64],
                v32[0][:].rearrange("p (c d) -> p c d", c=C))
            nc.vector.tensor_copy(
                vk1wv[:, :, 64:128],
                k32[0][:].rearrange("p (c d) -> p c d", c=C))
            nc.gpsimd.memset(vk1wv[:, :, 128:129], 1.0)
            VK1W[0] = vk1w
        # stages ~23/26us: b1 k dup-casts per half (k1a ~22, k1b ~25.5)
        kbv1 = vk1s[1][:].rearrange("p (c w) -> p c w", c=C)
        for half in range(2):
            with tc.tile_wait_until(0.023 + 0.003 * half):
                for hh in range(2):
                    nc.vector.tensor_copy(
                        kbv1[:, half * CH:(half + 1) * CH,
                             hh * 64:(hh + 1) * 64],
                        k32[1][:, half * CH * 64:(half + 1) * CH * 64]
                            .rearrange("p (c d) -> p c d", c=CH))
        # stage ~18us: vk1w b1 (needs v1)
        with tc.tile_wait_until(0.018):
            vk1w = vk1wp.tile([128, C * 132], BF16, tag="vk1w", name="vk1w1")
            vk1wv = vk1w[:].rearrange("p (c w) -> p c w", c=C)
            nc.vector.tensor_copy(
                vk1wv[:, :, 0:64],
                v32[1][:].rearrange("p (c d) -> p c d", c=C))
            nc.vector.tensor_copy(
                vk1wv[:, :, 64:128],
                k32[1][:].rearrange("p (c d) -> p c d", c=C))
            nc.gpsimd.memset(vk1wv[:, :, 128:129], 1.0)
            VK1W[1] = vk1w

        # ---------------- prefix: scoresT/exp + wv/wk/Z ----------------
        T2 = {}

        def tpair(src_ap, dst_ap):
            # PE transpose of two 128x128 bf16 chunks through one PSUM bank
            # (2nd write lands on still-pending bytes -> plain overwrite),
            # then one DVE copy out to SBUF.
            pt = tpsp.tile([128, 256], BF16, tag="tp")
            nc.tensor.matmul(pt[:, 0:128], src_ap[:, 0:128], ident16[:],
                             is_transpose=True, start=True, stop=False)
            nc.tensor.matmul(pt[:, 128:256], src_ap[:, 128:256], ident16[:],
                             is_transpose=True, start=False, stop=True)
            nc.vector.tensor_copy(dst_ap[:, 0:256], pt[:])

        def prefix(b):
            et = etp.tile([128, C * Q], BF16, tag="et", name=f"et{b}")
            ET[b] = et
            psw = {}
            for t in range(T):
                psw[t] = wvkpsp.tile([128, 132], FP32, tag="psw",
                                     name=f"psw{b}{t}")
            # Q^T on the (otherwise idle) PE, then first K pair (prologue)
            tpair(qbps[b][:], QT[b][:])
            tpair(vk1s[b][:, 0:256], VKT[b][:, 0:256])
            for c2 in range(C // 2):
                if c2 + 1 < C // 2:
                    tpair(vk1s[b][:, (c2 + 1) * 256:(c2 + 2) * 256],
                          VKT[b][:, (c2 + 1) * 256:(c2 + 2) * 256])
                pssc = scpsp.tile([128, 2 * Q], FP32, tag="pssc")
                for h in range(2):
                    c = 2 * c2 + h
                    # full-128 contraction via the [K^T;K^T]/[Q^T;Q^T]
                    # dup operands -> 2x score, absorbed in exp scale
                    nc.tensor.matmul(
                        pssc[:, h * Q:(h + 1) * Q],
                        VKT[b][:, c * 128:(c + 1) * 128],
                        QT[b][:, :],
                        start=True, stop=True)
                nc.scalar.activation(et[:, c2 * 2 * Q:(c2 + 1) * 2 * Q],
                                     pssc[:], AF.Exp, scale=SCALE / 2)
                for h in range(2):
                    c = 2 * c2 + h
                    for t in range(T):
                        nc.tensor.matmul(
                            psw[t][:, 0:129],
                            et[:, c * Q + t * 128: c * Q + t * 128 + 128],
                            VK1W[b][:, c * 132:c * 132 + 129],
                            start=(c == 0), stop=(c == C - 1))
            # Vdup on ACT (feeds DVE M-builds)
            vd = vdupp.tile([128, C * 128], BF16, tag="vdup")
            vk1wv_b = VK1W[b][:].rearrange("p (c w) -> p c w", c=C)
            nc.scalar.activation(
                vd[:].rearrange("p (c i e) -> p c i e", c=C, i=64),
                vk1wv_b[:, :, 0:64].unsqueeze(3).broadcast_to((128, C, 64, 2)),
                AF.Copy)
            VD[b] = vd
            for t in range(T):
                wvk = wvkp.tile([128, 132], FP32, tag="wvk")
                nc.scalar.activation(wvk[:, 0:129], psw[t][:, 0:129], AF.Copy)
                rq0 = smallp.tile([128, 1], FP32, tag="rq0")
                nc.vector.reciprocal(rq0[:], wvk[:, 128:129])
                rq1 = smallp.tile([128, 1], FP32, tag="rq1")
                nc.vector.tensor_scalar_mul(rq1[:], rq0[:], SCALE)
                RQ1[(b, t)] = rq1
                # wvp = -wvE/Z (bf16), wkp = wkE (bf16)
                wvp = smallp.tile([128, 64], BF16, tag="wvp")
                nc.vector.tensor_scalar(wvp[:], wvk[:, 0:64], rq0[:],
                                        -1.0, op0=ALU.mult, op1=ALU.mult)
                wkp = smallp.tile([128, 64], BF16, tag="wkp")
                nc.vector.tensor_copy(wkp[:], wvk[:, 64:128])
                # pair-dup of wvp on ACT
                wvpd = smallp.tile([128, 128], BF16, tag="wvpd")
                nc.scalar.activation(
                    wvpd[:].rearrange("p (i e) -> p i e", e=2),
                    wvp[:].unsqueeze(2).broadcast_to((128, 64, 2)),
                    AF.Copy)
                WVP[(b, t)] = wvpd
                WKP[(b, t)] = wkp
                if b == 0:
                    # ident-injected groups (b0 hq0/hq1) need T2 in SBUF;
                    # all later groups get T2 DVE-preloaded into PSUM
                    t2 = t2p.tile([128, 2 * 1024], BF16, tag="t2")
                    nc.vector.tensor_mul(
                        t2[:].rearrange("p (i j e) -> p i j e", i=32, j=32),
                        wvpd[:, 0:64].rearrange("p (i e) -> p i e", e=2)
                            .unsqueeze(2).broadcast_to((128, 32, 32, 2)),
                        wkp[:].rearrange("p (j e) -> p j e", e=2)
                            .unsqueeze(1).broadcast_to((128, 32, 32, 2)))
                    T2[(b, t)] = t2

        # ---------------- term1 ----------------
        def term1_group(b, hq, pool, preload):
            ps = {}
            for t in range(T):
                for j in range(2):
                    ps[(t, j)] = pool.tile(
                        [128, 512], FP32, tag="t1ps",
                        name=f"t1ps_{b}_{hq}_{t}_{j}")
                    if preload:
                        i0 = (hq * 16 + j * 8) * 2
                        nc.vector.tensor_mul(
                            ps[(t, j)][:].rearrange(
                                "p (i j2 e) -> p i j2 e", i=8, j2=32),
                            WVP[(b, t)][:, i0:i0 + 16]
                                .rearrange("p (i e) -> p i e", e=2)
                                .unsqueeze(2)
                                .broadcast_to((128, 8, 32, 2)),
                            WKP[(b, t)][:]
                                .rearrange("p (j2 e) -> p j2 e", e=2)
                                .unsqueeze(1)
                                .broadcast_to((128, 8, 32, 2)))
            for c in range(C):
                # M chunk on DVE (2x mode via pair-dup)
                m = mp.tile([128, IQ * 64], BF16, tag="m")
                nc.vector.tensor_mul(
                    m[:].rearrange("p (i j e) -> p i j e", i=IQ, j=32),
                    VD[b][:, c * 128 + hq * 32: c * 128 + (hq + 1) * 32]
                        .rearrange("p (i e) -> p i e", e=2)
                        .unsqueeze(2).broadcast_to((128, IQ, 32, 2)),
                    VK1[b][:, c * 128 + 64:(c + 1) * 128]
                        .rearrange("p (j e) -> p j e", e=2)
                        .unsqueeze(1).broadcast_to((128, IQ, 32, 2)))
                for t in range(T):
                    lhsT = ET[b][:, c * Q + t * 128: c * Q + t * 128 + 128]
                    for j in range(2):
                        nc.tensor.matmul(
                            ps[(t, j)][:], lhsT,
                            m[:, j * 512:(j + 1) * 512],
                            start=(c == 0 and not preload),
                            stop=(c == C - 1 and preload),
                            skip_group_check=preload)
            for t in range(T):
                for j in range(2):
                    if not preload:
                        nc.tensor.matmul(
                            ps[(t, j)][:], ident16[:],
                            T2[(b, t)][:, hq * 1024 + j * 512:
                                       hq * 1024 + (j + 1) * 512],
                            start=False, stop=True)
                    o = outsp.tile([128, 512], FP32, tag="outs")
                    nc.scalar.activation(o[:], ps[(t, j)][:], AF.Copy,
                                         scale=RQ1[(b, t)][:])
                    nc.sync.dma_start(
                        out_ext[b, t * 128:(t + 1) * 128,
                                hq * 1024 + j * 512:
                                hq * 1024 + (j + 1) * 512],
                        o[:])

        prefix(0)
        prefix(1)
        pfx.close()
        t1psp = stack.enter_context(
            tc.tile_pool(name="t1ps", bufs=8, space="PSUM"))
        for b in range(BB):
            for hq in range(NQ):
                term1_group(b, hq, t1psp,
                            preload=not (b == 0 and hq <= 1))
    return nc


_SPLITTABLE = {
    "InstDrain", "InstMatmult", "InstLdweights", "InstActivation",
    "InstTensorTensor", "InstTensorCopy", "InstTensorScalarPtr",
    "InstReciprocal", "InstMemset", "InstPartitionBroadcast",
    "InstTensorReduce", "InstNoOp", "InstTensorScalarAffineSelect",
    "InstEventSemaphore",
}


def fix_drain_waits(nc, max_waits=1):
    """This walrus build supports only `max_waits` sem-waits per instruction;
    move the excess onto preceding same-engine NOPs (kernel-graph post-pass).
    DMA instructions: queue-side DMA sem waits stay on the DMA (FIFO
    semantics), compute-engine waits are hoisted onto the issuing engine."""
    def emit_nops(waits, engine, new_insts):
        for cs in range(0, len(waits), max_waits):
            chunk = waits[cs:cs + max_waits]
            nop = mybir.InstNoOp(
                name=nc.get_next_instruction_name(), ins=[], outs=[],
                engine=engine,
                sync_info=mybir.SyncInfo(on_wait=list(chunk), on_update=[]),
            )
            new_insts.append(nop)

    for fn in nc.m.functions:
        for bb in fn.blocks:
            new_insts = []
            for inst in bb.instructions:
                w = inst.sync_info.on_wait if inst.sync_info else None
                if w and len(w) > max_waits:
                    nm = type(inst).__name__
                    if nm in _SPLITTABLE:
                        emit_nops(w[max_waits:], inst.engine, new_insts)
                        inst.sync_info.on_wait = list(w[:max_waits])
                    elif nm in ("InstDMACopy", "InstDmaTransposeAnt"):
                        dma_w = [s for s in w if "DMA" in (s.ant_name or "")]
                        other = [s for s in w if "DMA" not in (s.ant_name or "")]
                        keep = dma_w[:max_waits]
                        hoist = other + dma_w[max_waits:]
                        if not keep:
                            keep = [hoist.pop(0)]
                        emit_nops(hoist, inst.engine, new_insts)
                        inst.sync_info.on_wait = list(keep)
                new_insts.append(inst)
            bb.instructions = new_insts


_CACHED = {}


def _get_nc():
    if "nc" not in _CACHED:
        nc = bass.Bass()
        build(nc)
        fix_drain_waits(nc)
        _CACHED["nc"] = nc
    return _CACHED["nc"]


def kernel(query, keys, values):
    from concourse.bass_utils import run_bass_kernel_spmd

    query = np.ascontiguousarray(query, dtype=np.float32)
    keys = np.ascontiguousarray(keys, dtype=np.float32)
    values = np.ascontiguousarray(values, dtype=np.float32)
    nc = _get_nc()
    in_maps = [
        {
            "query": query[i * BB:(i + 1) * BB],
            "keys": keys[i * BB:(i + 1) * BB],
            "values": values[i * BB:(i + 1) * BB],
        }
        for i in range(NCORES)
    ]
    res = run_bass_kernel_spmd(nc, in_maps, core_ids=list(range(NCORES)))
    out = np.concatenate([r["out"].reshape(BB, Q, D, D) for r in res.results], axis=0)
    return out


# revision 31
# speedup vs baseline: 1.0243x; 1.0243x over previous
"""Attention-Jacobian kernel on 8 TRN2 NeuronCores (batch-sharded SPMD).

Full problem: query (16,256,64), keys (16,2048,64), values (16,2048,64)
-> out (16,256,64,64), out[b,q,i,j] = d attn_out[b,q,i] / d query[b,q,j]:
   scale * (sum_s a[q,s] v[s,i] k[s,j] - wv[q,i] wk[q,j])

Sharding: batch dim 16 -> 8 cores x 2 batches, pure data parallel.

Per-core algorithm (s-major, all heavy matmuls bf16 at N=512):
  - K^T/Q^T via DMA-xbar transposes of bf16 [K|K]/[Q|Q] dup tiles; scores
    run full-128 contraction (2x score absorbed into the exp scale)
  - exp on ACT -> ET bf16 (unnormalized; randn keeps scores ~N(0,1))
  - Z rides as a ones-column in the [V|K|1] rhs of the wv/wk accumulation;
    normalization folds into the PSUM->SBUF out-copy (ACT scale=SCALE/Z)
  - M[s, i*64+j] = V[s,i]*K[s,j] on DVE (pair-dup 2x_1P mode)
  - term1: PE c-major accumulation, lhsT = ET chunks, rhs = M chunks
  - term2: first two groups (PSUM bank first-use) via identity matmul of
    T2 = (-wvE/Z) x wkE; all later groups get T2 DVE-preloaded into PSUM
    and accumulate onto it with start=False
  - emission order prefix(b0) -> term1(b0,hq0) -> prefix(b1) -> rest, so
    the PE never queues exp-b1-gated work ahead of ready term1-b0 work
"""
import math
import numpy as np
import concourse.bass as bass
import concourse.tile as tile
from concourse import mybir
from concourse.masks import make_identity

FP32 = mybir.dt.float32
BF16 = mybir.dt.bfloat16
AF = mybir.ActivationFunctionType
ALU = mybir.AluOpType

NCORES = 8
B, Q, S, D = 16, 256, 2048, 64
BB = B // NCORES
SCALE = 1.0 / math.sqrt(D)

C = S // 128          # s-chunks (16)
T = Q // 128          # q-tiles (2)
NQ = 4                # i-quarters
IQ = D // NQ          # i per quarter (16)


def build(nc):
    from contextlib import ExitStack

    q_ext = nc.declare_dram_parameter("query", [BB, Q, D], FP32, isOutput=False)
    k_ext = nc.declare_dram_parameter("keys", [BB, S, D], FP32, isOutput=False)
    v_ext = nc.declare_dram_parameter("values", [BB, S, D], FP32, isOutput=False)
    out_ext = nc.declare_dram_parameter("out", [BB, Q, D * D], FP32, isOutput=True)

    with tile.TileContext(nc) as tc, ExitStack() as stack:
        ep = lambda name, bufs, **kw: stack.enter_context(
            tc.tile_pool(name=name, bufs=bufs, **kw))
        constp = ep("const", 1)
        kv32p = ep("kv32", 4)
        q32p = ep("q32", 2)
        vk1p = ep("vk1", 2)
        vk1wp = ep("vk1w", 2)
        vktp = ep("vkt", 2)
        qbpp = ep("qbp", 2)
        qtp = ep("qt", 2)
        etp = ep("et", 2)
        vdupp = ep("vdup", 2)
        wvkp = ep("wvk", 2)
        smallp = ep("small", 4)
        t2p = ep("t2", 2)
        mp = ep("m", 12)
        outsp = ep("outs", 6)

        ident32 = constp.tile([128, 128], FP32, tag="id32")
        make_identity(nc, ident32[:])
        ident16 = constp.tile([128, 128], BF16, tag="id16")
        nc.vector.tensor_copy(ident16[:], ident32[:])

        VK1, VK1W, VKT, QT, ET, VD = {}, {}, {}, {}, {}, {}
        WVP, WKP, RQ1 = {}, {}, {}

        # PSUM: scps(3 banks) + wvkps(part bank) + t1pspA(4 banks) coexist;
        # after pfx closes, t1pspB takes the freed banks.
        pfx = ExitStack()
        wmpsp = pfx.enter_context(tc.tile_pool(name="wmps", bufs=1, space="PSUM"))
        scpsp = pfx.enter_context(tc.tile_pool(name="scps", bufs=3, space="PSUM"))
        wvkpsp = pfx.enter_context(tc.tile_pool(name="wvkps", bufs=2, space="PSUM"))

        # ---------------- loads + casts + transposes ----------------
        # sync hwdge: q0, q1, k0a, k0b, qt0T, vkt0aT, vkt0bT, then staged
        # k1a, k1b, qt1T, vkt1T, then out DMAs. gpsimd swdge: v0, v1 only.
        CH = C // 2
        k32, v32, qq32, vk1s = {}, {}, {}, {}
        for b in range(BB):
            qq32[b] = q32p.tile([128, T * 64], FP32, tag="q32", name=f"qq{b}")
            k32[b] = kv32p.tile([128, C * 64], FP32, tag="k32", name=f"k32_{b}")
            v32[b] = kv32p.tile([128, C * 64], FP32, tag="v32", name=f"v32_{b}")
            kb = vk1p.tile([128, C * 128], BF16, tag="kb", name=f"kb{b}")
            vk1s[b] = kb
            VK1[b] = kb
            VKT[b] = vktp.tile([128, C * 128], BF16, tag="ktb", name=f"ktb{b}")
            QT[b] = qtp.tile([128, T * 128], BF16, tag="qt", name=f"qt{b}")
        qbps = {}
        with tc.high_priority():
            # HAM warmup on a memset tile: no DVE dependency; the warmup
            # PSUM tile shares the scps ring (same shape as pssc)
            z16 = constp.tile([128, 128], BF16, tag="z16")
            nc.gpsimd.memset(z16[:], 0.0)
            wm = wmpsp.tile([128, 128], FP32, tag="wm")
            for r in range(36):
                nc.tensor.matmul(wm[:], z16[:], z16[:], start=True, stop=True)
            for b in range(BB):
                nc.sync.dma_start(
                    qq32[b][:].rearrange("p (t d) -> p t d", t=T),
                    q_ext[b].rearrange("(t p) d -> p t d", p=128))
            for half in range(2):
                nc.sync.dma_start(
                    k32[0][:, half * CH * 64:(half + 1) * CH * 64]
                        .rearrange("p (c d) -> p c d", c=CH),
                    k_ext[0][half * CH * 128:(half + 1) * CH * 128]
                        .rearrange("(c p) d -> p c d", p=128))
            nc.gpsimd.dma_start(
                v32[0][:].rearrange("p (c d) -> p c d", c=C),
                v_ext[0].rearrange("(c p) d -> p c d", p=128))
            qbp = qbpp.tile([128, T * 128], BF16, tag="qbp", name="qbp0")
            for t in range(T):
                for hh in range(2):
                    nc.vector.tensor_copy(
                        qbp[:, t * 128 + hh * 64:t * 128 + (hh + 1) * 64],
                        qq32[0][:, t * 64:(t + 1) * 64])
            qbps[0] = qbp
            nc.sync.dma_start_transpose(
                QT[0][:].rearrange("p (t f) -> p t f", t=T), qbps[0][:])
            kbv0 = vk1s[0][:].rearrange("p (c w) -> p c w", c=C)
            for half in range(2):
                for hh in range(2):
                    nc.vector.tensor_copy(
                        kbv0[:, half * CH:(half + 1) * CH,
                             hh * 64:(hh + 1) * 64],
                        k32[0][:, half * CH * 64:(half + 1) * CH * 64]
                            .rearrange("p (c d) -> p c d", c=CH))
                nc.sync.dma_start_transpose(
                    VKT[0][:, half * CH * 128:(half + 1) * CH * 128]
                        .rearrange("p (c f) -> p c f", c=CH),
                    vk1s[0][:, half * CH * 128:(half + 1) * CH * 128])
        nc.gpsimd.dma_start(
            v32[1][:].rearrange("p (c d) -> p c d", c=C),
            v_ext[1].rearrange("(c p) d -> p c d", p=128))
        # stage ~9us: qbp1 + qt1T
        with tc.tile_wait_until(0.009):
            qbp = qbpp.tile([128, T * 128], BF16, tag="qbp", name="qbp1")
            for t in range(T):
                for hh in range(2):
                    nc.vector.tensor_copy(
                        qbp[:, t * 128 + hh * 64:t * 128 + (hh + 1) * 64],
                        qq32[1][:, t * 64:(t + 1) * 64])
            qbps[1] = qbp
            nc.sync.dma_start_transpose(
                QT[1][:].rearrange("p (t f) -> p t f", t=T), qbps[1][:])
        # stage ~12us: k1 on sync (queues behind the b0 transposes), vk1w b0
        with tc.tile_wait_until(0.012):
            for half in range(2):
                nc.sync.dma_start(
                    k32[1][:, half * CH * 64:(half + 1) * CH * 64]
                        .rearrange("p (c d) -> p c d", c=CH),
                    k_ext[1][half * CH * 128:(half + 1) * CH * 128]
                        .rearrange("(c p) d -> p c d", p=128))
            vk1w = vk1wp.tile([128, C * 132], BF16, tag="vk1w", name="vk1w0")
            vk1wv = vk1w[:].rearrange("p (c w) -> p c w", c=C)
            nc.vector.tensor_copy(
                vk1wv[:, :, 0:64],
                v32[0][:].rearrange("p (c d) -> p c d", c=C))
            nc.vector.tensor_copy(
                vk1wv[:, :, 64:128],
                k32[0][:].rearrange("p (c d) -> p c d", c=C))
            nc.gpsimd.memset(vk1wv[:, :, 128:129], 1.0)
            VK1W[0] = vk1w
        # stage ~16us: b1 k casts + vkt1T
        with tc.tile_wait_until(0.016):
            kbv1 = vk1s[1][:].rearrange("p (c w) -> p c w", c=C)
            for hh in range(2):
                nc.vector.tensor_copy(
                    kbv1[:, :, hh * 64:(hh + 1) * 64],
                    k32[1][:].rearrange("p (c d) -> p c d", c=C))
            nc.sync.dma_start_transpose(
                VKT[1][:].rearrange("p (c f) -> p c f", c=C), vk1s[1][:])
        # stage ~18us: vk1w b1 (needs v1)
        with tc.tile_wait_until(0.018):
            vk1w = vk1wp.tile([128, C * 132], BF16, tag="vk1w", name="vk1w1")
            vk1wv = vk1w[:].rearrange("p (c w) -> p c w", c=C)
            nc.vector.tensor_copy(
                vk1wv[:, :, 0:64],
                v32[1][:].rearrange("p (c d) -> p c d", c=C))
            nc.vector.tensor_copy(
                vk1wv[:, :, 64:128],
                k32[1][:].rearrange("p (c d) -> p c d", c=C))
            nc.gpsimd.memset(vk1wv[:, :, 128:129], 1.0)
            VK1W[1] = vk1w

        # ---------------- prefix: scoresT/exp + wv/wk/Z ----------------
        T2 = {}

        def prefix(b):
            et = etp.tile([128, C * Q], BF16, tag="et", name=f"et{b}")
            ET[b] = et
            psw = {}
            for t in range(T):
                psw[t] = wvkpsp.tile([128, 132], FP32, tag="psw",
                                     name=f"psw{b}{t}")
            for c2 in range(C // 2):
                pssc = scpsp.tile([128, 2 * Q], FP32, tag="pssc")
                for h in range(2):
                    c = 2 * c2 + h
                    # full-128 contraction via the [K^T;K^T]/[Q^T;Q^T]
                    # dup operands -> 2x score, absorbed in exp scale
                    nc.tensor.matmul(
                        pssc[:, h * Q:(h + 1) * Q],
                        VKT[b][:, c * 128:(c + 1) * 128],
                        QT[b][:, :],
                        start=True, stop=True)
                nc.scalar.activation(et[:, c2 * 2 * Q:(c2 + 1) * 2 * Q],
                                     pssc[:], AF.Exp, scale=SCALE / 2)
                for h in range(2):
                    c = 2 * c2 + h
                    for t in range(T):
                        nc.tensor.matmul(
                            psw[t][:, 0:129],
                            et[:, c * Q + t * 128: c * Q + t * 128 + 128],
                            VK1W[b][:, c * 132:c * 132 + 129],
                            start=(c == 0), stop=(c == C - 1))
            # Vdup on ACT (feeds DVE M-builds)
            vd = vdupp.tile([128, C * 128], BF16, tag="vdup")
            vk1wv_b = VK1W[b][:].rearrange("p (c w) -> p c w", c=C)
            nc.scalar.activation(
                vd[:].rearrange("p (c i e) -> p c i e", c=C, i=64),
                vk1wv_b[:, :, 0:64].unsqueeze(3).broadcast_to((128, C, 64, 2)),
                AF.Copy)
            VD[b] = vd
            for t in range(T):
                wvk = wvkp.tile([128, 132], FP32, tag="wvk")
                nc.scalar.activation(wvk[:, 0:129], psw[t][:, 0:129], AF.Copy)
                rq0 = smallp.tile([128, 1], FP32, tag="rq0")
                nc.vector.reciprocal(rq0[:], wvk[:, 128:129])
                rq1 = smallp.tile([128, 1], FP32, tag="rq1")
                nc.vector.tensor_scalar_mul(rq1[:], rq0[:], SCALE)
                RQ1[(b, t)] = rq1
                # wvp = -wvE/Z (bf16), wkp = wkE (bf16)
                wvp = smallp.tile([128, 64], BF16, tag="wvp")
                nc.vector.tensor_scalar(wvp[:], wvk[:, 0:64], rq0[:],
                                        -1.0, op0=ALU.mult, op1=ALU.mult)
                wkp = smallp.tile([128, 64], BF16, tag="wkp")
                nc.vector.tensor_copy(wkp[:], wvk[:, 64:128])
                # pair-dup of wvp on ACT
                wvpd = smallp.tile([128, 128], BF16, tag="wvpd")
                nc.scalar.activation(
                    wvpd[:].rearrange("p (i e) -> p i e", e=2),
                    wvp[:].unsqueeze(2).broadcast_to((128, 64, 2)),
                    AF.Copy)
                WVP[(b, t)] = wvpd
                WKP[(b, t)] = wkp
                if b == 0:
                    # ident-injected groups (b0 hq0/hq1) need T2 in SBUF;
                    # all later groups get T2 DVE-preloaded into PSUM
                    t2 = t2p.tile([128, 2 * 1024], BF16, tag="t2")
                    nc.vector.tensor_mul(
                        t2[:].rearrange("p (i j e) -> p i j e", i=32, j=32),
                        wvpd[:, 0:64].rearrange("p (i e) -> p i e", e=2)
                            .unsqueeze(2).broadcast_to((128, 32, 32, 2)),
                        wkp[:].rearrange("p (j e) -> p j e", e=2)
                            .unsqueeze(1).broadcast_to((128, 32, 32, 2)))
                    T2[(b, t)] = t2

        # ---------------- term1 ----------------
        def term1_group(b, hq, pool, preload):
            ps = {}
            for t in range(T):
                for j in range(2):
                    ps[(t, j)] = pool.tile(
                        [128, 512], FP32, tag="t1ps",
                        name=f"t1ps_{b}_{hq}_{t}_{j}")
                    if preload:
                        i0 = (hq * 16 + j * 8) * 2
                        nc.vector.tensor_mul(
                            ps[(t, j)][:].rearrange(
                                "p (i j2 e) -> p i j2 e", i=8, j2=32),
                            WVP[(b, t)][:, i0:i0 + 16]
                                .rearrange("p (i e) -> p i e", e=2)
                                .unsqueeze(2)
                                .broadcast_to((128, 8, 32, 2)),
                            WKP[(b, t)][:]
                                .rearrange("p (j2 e) -> p j2 e", e=2)
                                .unsqueeze(1)
                                .broadcast_to((128, 8, 32, 2)))
            for c in range(C):
                # M chunk on DVE (2x mode via pair-dup)
                m = mp.tile([128, IQ * 64], BF16, tag="m")
                nc.vector.tensor_mul(
                    m[:].rearrange("p (i j e) -> p i j e", i=IQ, j=32),
                    VD[b][:, c * 128 + hq * 32: c * 128 + (hq + 1) * 32]
                        .rearrange("p (i e) -> p i e", e=2)
                        .unsqueeze(2).broadcast_to((128, IQ, 32, 2)),
                    VK1[b][:, c * 128 + 64:(c + 1) * 128]
                        .rearrange("p (j e) -> p j e", e=2)
                        .unsqueeze(1).broadcast_to((128, IQ, 32, 2)))
                for t in range(T):
                    lhsT = ET[b][:, c * Q + t * 128: c * Q + t * 128 + 128]
                    for j in range(2):
                        nc.tensor.matmul(
                            ps[(t, j)][:], lhsT,
                            m[:, j * 512:(j + 1) * 512],
                            start=(c == 0 and not preload),
                            stop=(c == C - 1 and preload),
                            skip_group_check=preload)
            for t in range(T):
                for j in range(2):
                    if not preload:
                        nc.tensor.matmul(
                            ps[(t, j)][:], ident16[:],
                            T2[(b, t)][:, hq * 1024 + j * 512:
                                       hq * 1024 + (j + 1) * 512],
                            start=False, stop=True)
                    o = outsp.tile([128, 512], FP32, tag="outs")
                    nc.scalar.activation(o[:], ps[(t, j)][:], AF.Copy,
                                         scale=RQ1[(b, t)][:])
                    nc.sync.dma_start(
                        out_ext[b, t * 128:(t + 1) * 128,
                                hq * 1024 + j * 512:
                                hq * 1024 + (j + 1) * 512],
                        o[:])

        prefix(0)
        prefix(1)
        pfx.close()
        t1psp = stack.enter_context(
            tc.tile_pool(name="t1ps", bufs=8, space="PSUM"))
        for b in range(BB):
            for hq in range(NQ):
                term1_group(b, hq, t1psp,
                            preload=not (b == 0 and hq <= 1))
    return nc


_SPLITTABLE = {
    "InstDrain", "InstMatmult", "InstLdweights", "InstActivation",
    "InstTensorTensor", "InstTensorCopy", "InstTensorScalarPtr",
    "InstReciprocal", "InstMemset", "InstPartitionBroadcast",
    "InstTensorReduce", "InstNoOp", "InstTensorScalarAffineSelect",
    "InstEventSemaphore",
}


def fix_drain_waits(nc, max_waits=1):
    """This walrus build supports only `max_waits` sem-waits per instruction;
    move the excess onto preceding same-engine NOPs (kernel-graph post-pass).
    DMA instructions: queue-side DMA sem waits stay on the DMA (FIFO
    semantics), compute-engine waits are hoisted onto the issuing engine."""
    def emit_nops(waits, engine, new_insts):
        for cs in range(0, len(waits), max_waits):
            chunk = waits[cs:cs + max_waits]
            nop = mybir.InstNoOp(
                name=nc.get_next_instruction_name(), ins=[], outs=[],
                engine=engine,
                sync_info=mybir.SyncInfo(on_wait=list(chunk), on_update=[]),
            )
            new_insts.append(nop)

    for fn in nc.m.functions:
        for bb in fn.blocks:
            new_insts = []
            for inst in bb.instructions:
                w = inst.sync_info.on_wait if inst.sync_info else None
                if w and len(w) > max_waits:
                    nm = type(inst).__name__
                    if nm in _SPLITTABLE:
                        emit_nops(w[max_waits:], inst.engine, new_insts)
                        inst.sync_info.on_wait = list(w[:max_waits])
                    elif nm in ("InstDMACopy", "InstDmaTransposeAnt"):
                        dma_w = [s for s in w if "DMA" in (s.ant_name or "")]
                        other = [s for s in w if "DMA" not in (s.ant_name or "")]
                        keep = dma_w[:max_waits]
                        hoist = other + dma_w[max_waits:]
                        if not keep:
                            keep = [hoist.pop(0)]
                        emit_nops(hoist, inst.engine, new_insts)
                        inst.sync_info.on_wait = list(keep)
                new_insts.append(inst)
            bb.instructions = new_insts


_CACHED = {}


def _get_nc():
    if "nc" not in _CACHED:
        nc = bass.Bass()
        build(nc)
        fix_drain_waits(nc)
        _CACHED["nc"] = nc
    return _CACHED["nc"]


def kernel(query, keys, values):
    from concourse.bass_utils import run_bass_kernel_spmd

    query = np.ascontiguousarray(query, dtype=np.float32)
    keys = np.ascontiguousarray(keys, dtype=np.float32)
    values = np.ascontiguousarray(values, dtype=np.float32)
    nc = _get_nc()
    in_maps = [
        {
            "query": query[i * BB:(i + 1) * BB],
            "keys": keys[i * BB:(i + 1) * BB],
            "values": values[i * BB:(i + 1) * BB],
        }
        for i in range(NCORES)
    ]
    res = run_bass_kernel_spmd(nc, in_maps, core_ids=list(range(NCORES)))
    out = np.concatenate([r["out"].reshape(BB, Q, D, D) for r in res.results], axis=0)
    return out
